# revision 35
# baseline (speedup 1.0000x reference)
"""Trainium2 Bass kernel for nn_RefineLoss (Sobel-gradient refine loss).

Math: with gm=sqrt(|grad g|^2+eps), pm=sqrt(|grad p|^2+eps), dot=grad_g.grad_p:
  Lrefine = 0.5*(Lcos + Lmag)*mask,  Lcos = pm - |dot|/gm,
  Lmag = relu(1.5*gm - pm)  =>  Lcos + Lmag = max(1.5*gm, pm) - |dot|/gm.
Output = mean over the full tensor.

Mapping (per core: 2 images):
  - 3x3 separable Sobel convs run fully on TensorE: vertical filter as banded
    lhsT matmuls, horizontal taps via column-shifted rhs APs accumulated in
    PSUM. The g-side weights carry the 1.5 factor so 1.5*gm comes for free.
  - 128-row strips with 2-row overlap (stride 126); final 15 rows of both
    images packed into one 32-partition strip.
  - gx|gy (and px|py) land stacked in one [K,2048] PSUM tile so each
    elementwise pass covers both with one wide instruction.
  - sqrt/reciprocal via exp/ln on ScalarE (one table set, high precision):
    gm' = exp(0.5*ln(g2e)), 1/gm = exp(-0.5*ln(g2e)).
  - Per-strip masked partial sums accumulate per-partition into a [128,17]
    f32 tile (scalar_tensor_tensor accum); host does validity masking and the
    final float64 reduction.
"""

import functools

import numpy as np
import ml_dtypes

import concourse.bass as bass
import concourse.mybir as mybir
from concourse.tile import TileContext
from concourse.vector_clock import ScopedClock
from concourse.bass_utils import run_bass_kernel_spmd
from concourse.mybir import AluOpType as Op, ActivationFunctionType as Act

F32 = mybir.dt.float32
F32R = mybir.dt.float32r
BF16 = mybir.dt.bfloat16

H = W = 1024
N_IMGS = 16
N_CORES = 8
IMGS_PER_CORE = 2
EPS = 1e-6
ALPHA = 1.5
LN15 = float(np.log(1.5))

N_COLS = 17  # 2 imgs * 8 strips + 1 packed tail


def _patch_drain_split():
    """walrus in this container accepts only ONE sem-wait per instruction.
    Split every multi-wait instruction into single-wait NoOps emitted just
    before it on the same engine."""
    if getattr(TileContext, "_drain_split_patched", False):
        return

    def _patched(self, tick_clock, wait_clock):
        drain_inst = self.nc.sync.drain()
        wait_clock.add_sem_waits(
            drain_inst.ins, ScopedClock({None: tick_clock.global_clock})
        )
        si = drain_inst.ins.sync_info
        waits = list(si.on_wait or [])
        if len(waits) > 1:
            si.on_wait = waits[:1]
            for w in waits[1:]:
                nop = self.nc.sync.nop()
                nsi = nop.ins.sync_info
                if nsi is None:
                    nop.ins.sync_info = mybir.SyncInfo(on_wait=[w], on_update=[])
                else:
                    nsi.on_wait = [w]
        self.nc.all_engine_barrier()
        assert self.sems is not None
        popped = self.nc._tile_sem_poison_stack.pop()
        assert popped is self._sem_poison
        self.nc.clear_and_free_semaphores(list(self.sems.allocated().values()))
        self.nc.all_engine_barrier()

    TileContext._drain_and_barrier = _patched

    _orig_lower = TileContext._lower_ordered_insts

    def _lower_split(self, ordered):
        for bbname, insts in ordered.items():
            new = []
            for inst in insts:
                si = getattr(inst, "sync_info", None)
                waits = list(si.on_wait) if (si is not None and si.on_wait) else []
                if len(waits) > 1:
                    for i, w in enumerate(waits[:-1]):
                        new.append(mybir.InstNoOp(
                            name=f"{inst.name}_sw{i}",
                            sync_info=mybir.SyncInfo(on_wait=[w], on_update=[]),
                            bass_nofuse=True,
                            engine=inst.engine,
                        ))
                    si.on_wait = waits[-1:]
                new.append(inst)
            insts[:] = new
        return _orig_lower(self, ordered)

    TileContext._lower_ordered_insts = _lower_split
    TileContext._drain_split_patched = True


def _banded(K, M, entries):
    a = np.zeros((K, M), dtype=np.float32)
    for m in range(M):
        for off, wgt in entries:
            k = m + off
            if 0 <= k < K:
                a[k, m] = wgt
    return a


def _make_matrices():
    """Vertical-filter lhsT matrices (bf16), per strip kind.

    top   (s=0): partition k = img row k;      out m = row m,    valid m 0..126
    mid (s>=1): partition k = img row r0-1+k;  out m = row r0+m, valid m 0..125
    tail: two [16,16] diag blocks; partition kb = row 1008+kb, out mb = row
          1009+mb, valid mb 0..14.
    Returns {kind: {"sm": smooth, "df": diff}} as float32 [K, M].
    """
    mats = {}
    sm = _banded(128, 128, [(-1, 1.0), (0, 2.0), (1, 1.0)])
    df = _banded(128, 128, [(-1, 1.0), (1, -1.0)])
    sm[:, 127] = 0.0
    df[:, 127] = 0.0
    mats["top"] = (sm, df)
    sm = _banded(128, 128, [(0, 1.0), (1, 2.0), (2, 1.0)])
    df = _banded(128, 128, [(0, 1.0), (2, -1.0)])
    sm[:, 126:] = 0.0
    df[:, 126:] = 0.0
    mats["mid"] = (sm, df)
    smb = _banded(16, 16, [(0, 1.0), (1, 2.0), (2, 1.0)])
    dfb = _banded(16, 16, [(0, 1.0), (2, -1.0)])
    smb[:, 15] = 0.0
    dfb[:, 15] = 0.0
    z = np.zeros((16, 16), np.float32)
    mats["tail"] = (np.block([[smb, z], [z, smb]]), np.block([[dfb, z], [z, dfb]]))
    return mats


def _shift_cols(a, d, n=512, width=W):
    """rhs col window [a+d, a+d+n) clipped to [0,width) -> (s0, s1, j0, j1)."""
    j0 = max(0, -(a + d))
    j1 = min(n, width - (a + d))
    return a + d + j0, a + d + j1, j0, j1


@functools.lru_cache(maxsize=1)
def build_nc():
    _patch_drain_split()
    nc = bass.Bass()

    g = nc.dram_tensor("g", [IMGS_PER_CORE, H, W], F32, kind="ExternalInput")
    p = nc.dram_tensor("p", [IMGS_PER_CORE, H, W], F32, kind="ExternalInput")
    m = nc.dram_tensor("mk", [IMGS_PER_CORE, H, W], F32, kind="ExternalInput")
    out = nc.dram_tensor("acc_out", [128, N_COLS], F32, kind="ExternalOutput")

    base_mats = _make_matrices()

    with TileContext(nc) as tc:
        with (
            tc.tile_pool(name="const", bufs=1) as constp,
            tc.tile_pool(name="accp", bufs=1) as accp,
            tc.tile_pool(name="inp", bufs=3) as inp,
            tc.tile_pool(name="psum", bufs=2, space="PSUM") as psp,
            tc.tile_pool(name="work", bufs=2) as wk,
        ):
            # constants: g-side weights carry the 1.5 factor (amg = 1.5*gm),
            # p-side unscaled; packed into one tensor -> single DMA
            packed = []
            keys = []
            for kind, (sm, df) in base_mats.items():
                for side, scl in (("g", ALPHA), ("p", 1.0)):
                    for nm, arr in (
                        ("smP", scl * sm), ("smN", -scl * sm),
                        ("df", scl * df), ("df2", 2.0 * scl * df),
                    ):
                        keys.append(f"{kind}_{side}_{nm}")
                        a = np.zeros((128, 128), np.float32)
                        a[:arr.shape[0], :arr.shape[1]] = arr
                        packed.append(a)
            packed_np = np.ascontiguousarray(
                np.concatenate(packed, axis=1).astype(ml_dtypes.bfloat16))
            cdram = nc.inline_tensor(packed_np, name="c_all")
            call = constp.tile(list(packed_np.shape), BF16, name="ct_all", tag="ct_all")
            nc.sync.dma_start(out=call[:], in_=cdram[:])
            consts = {}
            for i, key in enumerate(keys):
                consts[key] = call[:, 128 * i:128 * i + 128]

            acc = accp.tile([128, N_COLS], F32, name="acc", tag="acc")
            epsbg = accp.tile([128, 1], F32, name="epsbg", tag="epsbg")
            nc.vector.memset(epsbg[:], ALPHA * ALPHA * EPS)
            epsbp = accp.tile([128, 1], F32, name="epsbp", tag="epsbp")
            nc.vector.memset(epsbp[:], EPS)
            absmask = accp.tile([128, 1024], BF16, name="absmask", tag="absmask")
            nc.vector.memset(absmask[:].bitcast(mybir.dt.uint16), 0x7FFF)

            def do_strip(kind, K, gt, pt, mt, col):
                sqg = wk.tile([K, 2048], BF16, name="sqg", tag="sqg", bufs=3)
                bpxy = wk.tile([K, 2048], BF16, name="bpxy", tag="bpxy", bufs=3)
                prods = wk.tile([K, 2048], BF16, name="prods", tag="prods", bufs=3)
                r2 = lambda ap: ap.rearrange("p (two n) -> p two n", two=2)
                for c in (0, 1):
                    a = 512 * c
                    GXYc = psp.tile([K, 1024], F32, name="GXYc", tag="psG")
                    PXYc = psp.tile([K, 1024], F32, name="PXYc", tag="psP")
                    for side, src, ps in (("g", gt, GXYc), ("p", pt, PXYc)):
                        # x-grad -> ps[:, 0:512]; shifts L(+w), R(-w)
                        shifts_x = [(+1, f"{kind}_{side}_smN"), (-1, f"{kind}_{side}_smP")]
                        shifts_x.sort(
                            key=lambda t_: _shift_cols(a, t_[0])[3] - _shift_cols(a, t_[0])[2],
                            reverse=True)
                        for i, (d, mk_) in enumerate(shifts_x):
                            s0, s1, j0, j1 = _shift_cols(a, d)
                            nc.tensor.matmul(
                                ps[:, j0:j1],
                                lhsT=consts[mk_][0:K, 0:K],
                                rhs=src[:, s0:s1],
                                start=(i == 0), stop=(i == len(shifts_x) - 1))
                        # y-grad -> ps[:, 512:1024]
                        shifts_y = [(0, f"{kind}_{side}_df2"),
                                    (-1, f"{kind}_{side}_df"), (+1, f"{kind}_{side}_df")]
                        for i, (d, mk_) in enumerate(shifts_y):
                            s0, s1, j0, j1 = _shift_cols(a, d)
                            nc.tensor.matmul(
                                ps[:, 512 + j0:512 + j1],
                                lhsT=consts[mk_][0:K, 0:K],
                                rhs=src[:, s0:s1],
                                start=(i == 0), stop=(i == len(shifts_y) - 1))

                    # extraction into wide halves-layout tiles via 3D APs:
                    # g extracted raw (for products) + squared; p squared
                    # straight from PSUM (raw p never needs to reach SBUF)
                    nc.scalar.activation(r2(sqg)[:, :, a:a + 512], r2(GXYc), Act.Square)
                    nc.scalar.copy(r2(bpxy)[:, :, a:a + 512], r2(PXYc))
                    nc.vector.tensor_tensor(
                        r2(prods)[:, :, a:a + 512], r2(GXYc),
                        r2(bpxy)[:, :, a:a + 512], Op.mult)

                # ---- wide SBUF elementwise ----
                sqp = wk.tile([K, 2048], BF16, name="sqp", tag="sqp")
                nc.gpsimd.tensor_tensor(sqp[:, 0:1024], bpxy[:, 0:1024], bpxy[:, 0:1024], Op.mult)
                nc.vector.tensor_tensor(sqp[:, 1024:2048], bpxy[:, 1024:2048], bpxy[:, 1024:2048], Op.mult)

                e2 = wk.tile([K, 2048], BF16, name="e2", tag="e2")
                nc.vector.tensor_tensor(
                    e2[:, 0:1024], sqg[:, 0:1024], sqg[:, 1024:2048], Op.add)
                nc.vector.tensor_tensor(
                    e2[:, 1024:2048], sqp[:, 0:1024], sqp[:, 1024:2048], Op.add)

                lgp = wk.tile([K, 2048], F32, name="lgp", tag="lgp")
                nc.scalar.activation(lgp[:, 0:1024], e2[:, 0:1024], Act.Ln, bias=epsbg[0:K, :])
                nc.scalar.activation(lgp[:, 1024:2048], e2[:, 1024:2048], Act.Ln, bias=epsbp[0:K, :])
                mlg = wk.tile([K, 1024], F32, name="mlg", tag="mlg", bufs=3)
                nc.vector.tensor_tensor(mlg[:], lgp[:, 0:1024], lgp[:, 1024:2048], Op.max)
                mx = wk.tile([K, 1024], BF16, name="mx", tag="mx", bufs=3)
                nc.scalar.activation(mx[:], mlg[:], Act.Exp, scale=0.5)
                rg = wk.tile([K, 1024], BF16, name="rg", tag="rg", bufs=3)
                nc.scalar.activation(rg[:], lgp[:, 0:1024], Act.Exp, scale=-0.5)

                dotv = wk.tile([K, 1024], BF16, name="dotv", tag="dotv", bufs=3)
                nc.gpsimd.tensor_tensor(dotv[:], prods[:, 0:1024], prods[:, 1024:2048], Op.add)
                adot = wk.tile([K, 1024], BF16, name="adot", tag="adot", bufs=3)
                nc.vector.tensor_tensor(
                    adot[:].bitcast(mybir.dt.uint16), dotv[:].bitcast(mybir.dt.uint16),
                    absmask[0:K, :].bitcast(mybir.dt.uint16), Op.bitwise_and)
                z0 = wk.tile([K, 1024], BF16, name="z0", tag="z0", bufs=3)
                nc.gpsimd.tensor_tensor(z0[:], adot[:], rg[:], Op.mult)
                tot = wk.tile([K, 1024], BF16, name="tot", tag="tot", bufs=3)
                nc.vector.tensor_tensor(tot[:], mx[:], z0[:], Op.subtract)

                dump = wk.tile([K, 1024], BF16, name="dump", tag="dump", bufs=3)
                nc.vector.scalar_tensor_tensor(
                    dump[:], tot[:], 1.0, mt[:], Op.mult, Op.mult,
                    accum_out=acc[0:K, col:col + 1])

            gt = inp.tile([32, W], BF16, name="gtt", tag="gtt")
            pt = inp.tile([32, W], BF16, name="ptt", tag="ptt")
            mt = inp.tile([32, W], F32, name="mtt", tag="mtt")
            with tc.high_priority():
                nc.gpsimd.memset(mt[:], 0)
                for img in range(IMGS_PER_CORE):
                    o = 16 * img
                    nc.gpsimd.dma_start(out=gt[o:o + 16, :], in_=g[img, 1008:1024, :])
                    nc.gpsimd.dma_start(out=pt[o:o + 16, :], in_=p[img, 1008:1024, :])
                    nc.sync.dma_start(out=mt[o:o + 15, :], in_=m[img, 1009:1024, :])
            do_strip("tail", 32, gt, pt, mt, 16)

            def mk_r0(s):
                return 0 if s == 0 else 127 + 126 * (s - 1)

            for img in range(IMGS_PER_CORE):
                for s0 in (0, 2, 4, 6):
                    # two strips per ~1MB DMA: partition p <- row 126*s + p
                    gt2 = inp.tile([128, 2 * W], BF16, name="gt2", tag="gt2")
                    pt2 = inp.tile([128, 2 * W], BF16, name="pt2", tag="pt2")
                    mt2 = inp.tile([128, 2 * W], F32, name="mt2", tag="mt2")
                    with tc.high_priority():
                        if img == 0 and s0 in (0, 2):
                            for ds in (0, 1):
                                for src_d, dst in ((g, gt2), (p, pt2)):
                                    ap = bass.AP(src_d, 126 * (s0 + ds) * W,
                                                 [[W, 128], [1, W]])
                                    nc.gpsimd.dma_start(
                                        out=dst[:, ds * W:(ds + 1) * W], in_=ap)
                        else:
                            for src_d, dst in ((g, gt2), (p, pt2)):
                                ap = bass.AP(src_d, img * H * W + 126 * s0 * W,
                                             [[W, 128], [126 * W, 2], [1, W]])
                                nc.gpsimd.dma_start(out=dst[:], in_=ap)
                    r0, r1 = mk_r0(s0), mk_r0(s0 + 1)
                    map_ = bass.AP(m, img * H * W + r0 * W,
                                   [[W, 128], [(r1 - r0) * W, 2], [1, W]])
                    nc.sync.dma_start(out=mt2[:], in_=map_)
                    for ds in (0, 1):
                        s = s0 + ds
                        kind = "top" if s == 0 else "mid"
                        do_strip(kind, 128,
                                 gt2[:, ds * W:(ds + 1) * W],
                                 pt2[:, ds * W:(ds + 1) * W],
                                 mt2[:, ds * W:(ds + 1) * W],
                                 img * 8 + s)


            nc.sync.dma_start(out=out[:], in_=acc[:])

    nc.finalize()
    return nc


def _valid_mask():
    v = np.zeros((128, N_COLS), dtype=bool)
    for img in range(IMGS_PER_CORE):
        base = img * 8
        v[0:127, base + 0] = True
        for s in range(1, 8):
            v[0:126, base + s] = True
    v[0:15, 16] = True
    v[16:31, 16] = True
    return v


def kernel(grayimg, pred, mask):
    g = np.ascontiguousarray(np.asarray(grayimg, dtype=np.float32).reshape(N_IMGS, H, W))
    p = np.ascontiguousarray(np.asarray(pred, dtype=np.float32).reshape(N_IMGS, H, W))
    mk = np.ascontiguousarray(np.asarray(mask, dtype=np.float32).reshape(N_IMGS, H, W))

    nc = build_nc()
    in_maps = []
    for c in range(N_CORES):
        sl = slice(c * IMGS_PER_CORE, (c + 1) * IMGS_PER_CORE)
        in_maps.append({"g": g[sl], "p": p[sl], "mk": mk[sl]})

    res = run_bass_kernel_spmd(nc, in_maps, core_ids=list(range(N_CORES)))

    vm = _valid_mask()
    total = 0.0
    for r in res.results:
        a = r["acc_out"].astype(np.float64)
        total += a[vm].sum()
    val = 0.5 * total / (N_IMGS * H * W)
    return np.float32(val)


# revision 44
# speedup vs baseline: 1.0386x; 1.0386x over previous
"""Trainium2 Bass kernel for nn_RefineLoss (Sobel-gradient refine loss).

Math: with gm=sqrt(|grad g|^2+eps), pm=sqrt(|grad p|^2+eps), dot=grad_g.grad_p:
  Lrefine = 0.5*(Lcos + Lmag)*mask,  Lcos = pm - |dot|/gm,
  Lmag = relu(1.5*gm - pm)  =>  Lcos + Lmag = max(1.5*gm, pm) - |dot|/gm.
Output = mean over the full tensor.

Mapping (per core: 2 images):
  - 3x3 separable Sobel convs run fully on TensorE: vertical filter as banded
    lhsT matmuls, horizontal taps via column-shifted rhs APs accumulated in
    PSUM. The g-side weights carry the 1.5 factor so 1.5*gm comes for free.
  - 128-row strips with 2-row overlap (stride 126); final 15 rows of both
    images packed into one 32-partition strip.
  - gx|gy (and px|py) land stacked in one [K,2048] PSUM tile so each
    elementwise pass covers both with one wide instruction.
  - sqrt/reciprocal via exp/ln on ScalarE (one table set, high precision):
    gm' = exp(0.5*ln(g2e)), 1/gm = exp(-0.5*ln(g2e)).
  - Per-strip masked partial sums accumulate per-partition into a [128,17]
    f32 tile (scalar_tensor_tensor accum); host does validity masking and the
    final float64 reduction.
"""

import functools

import numpy as np
import ml_dtypes

import concourse.bass as bass
import concourse.mybir as mybir
from concourse.tile import TileContext
from concourse.vector_clock import ScopedClock
from concourse.bass_utils import run_bass_kernel_spmd
from concourse.mybir import AluOpType as Op, ActivationFunctionType as Act

F32 = mybir.dt.float32
F32R = mybir.dt.float32r
BF16 = mybir.dt.bfloat16

H = W = 1024
N_IMGS = 16
N_CORES = 8
IMGS_PER_CORE = 2
EPS = 1e-6
ALPHA = 1.5
LN15 = float(np.log(1.5))

N_COLS = 17  # 2 imgs * 8 strips + 1 packed tail


def _patch_drain_split():
    """walrus in this container accepts only ONE sem-wait per instruction.
    Split every multi-wait instruction into single-wait NoOps emitted just
    before it on the same engine."""
    if getattr(TileContext, "_drain_split_patched", False):
        return

    def _patched(self, tick_clock, wait_clock):
        drain_inst = self.nc.sync.drain()
        wait_clock.add_sem_waits(
            drain_inst.ins, ScopedClock({None: tick_clock.global_clock})
        )
        si = drain_inst.ins.sync_info
        waits = list(si.on_wait or [])
        if len(waits) > 1:
            si.on_wait = waits[:1]
            for w in waits[1:]:
                nop = self.nc.sync.nop()
                nsi = nop.ins.sync_info
                if nsi is None:
                    nop.ins.sync_info = mybir.SyncInfo(on_wait=[w], on_update=[])
                else:
                    nsi.on_wait = [w]
        self.nc.all_engine_barrier()
        assert self.sems is not None
        popped = self.nc._tile_sem_poison_stack.pop()
        assert popped is self._sem_poison
        self.nc.clear_and_free_semaphores(list(self.sems.allocated().values()))
        self.nc.all_engine_barrier()

    TileContext._drain_and_barrier = _patched

    _orig_lower = TileContext._lower_ordered_insts

    def _lower_split(self, ordered):
        for bbname, insts in ordered.items():
            new = []
            for inst in insts:
                si = getattr(inst, "sync_info", None)
                waits = list(si.on_wait) if (si is not None and si.on_wait) else []
                if len(waits) > 1:
                    for i, w in enumerate(waits[:-1]):
                        new.append(mybir.InstNoOp(
                            name=f"{inst.name}_sw{i}",
                            sync_info=mybir.SyncInfo(on_wait=[w], on_update=[]),
                            bass_nofuse=True,
                            engine=inst.engine,
                        ))
                    si.on_wait = waits[-1:]
                new.append(inst)
            insts[:] = new
        return _orig_lower(self, ordered)

    TileContext._lower_ordered_insts = _lower_split
    TileContext._drain_split_patched = True


def _banded(K, M, entries):
    a = np.zeros((K, M), dtype=np.float32)
    for m in range(M):
        for off, wgt in entries:
            k = m + off
            if 0 <= k < K:
                a[k, m] = wgt
    return a


def _make_matrices():
    """Vertical-filter lhsT matrices (bf16), per strip kind.

    top   (s=0): partition k = img row k;      out m = row m,    valid m 0..126
    mid (s>=1): partition k = img row r0-1+k;  out m = row r0+m, valid m 0..125
    tail: two [16,16] diag blocks; partition kb = row 1008+kb, out mb = row
          1009+mb, valid mb 0..14.
    Returns {kind: {"sm": smooth, "df": diff}} as float32 [K, M].
    """
    mats = {}
    sm = _banded(128, 128, [(-1, 1.0), (0, 2.0), (1, 1.0)])
    df = _banded(128, 128, [(-1, 1.0), (1, -1.0)])
    sm[:, 127] = 0.0
    df[:, 127] = 0.0
    mats["top"] = (sm, df)
    sm = _banded(128, 128, [(0, 1.0), (1, 2.0), (2, 1.0)])
    df = _banded(128, 128, [(0, 1.0), (2, -1.0)])
    sm[:, 126:] = 0.0
    df[:, 126:] = 0.0
    mats["mid"] = (sm, df)
    smb = _banded(16, 16, [(0, 1.0), (1, 2.0), (2, 1.0)])
    dfb = _banded(16, 16, [(0, 1.0), (2, -1.0)])
    smb[:, 15] = 0.0
    dfb[:, 15] = 0.0
    z = np.zeros((16, 16), np.float32)
    mats["tail"] = (np.block([[smb, z], [z, smb]]), np.block([[dfb, z], [z, dfb]]))
    return mats


def _shift_cols(a, d, n=512, width=W):
    """rhs col window [a+d, a+d+n) clipped to [0,width) -> (s0, s1, j0, j1)."""
    j0 = max(0, -(a + d))
    j1 = min(n, width - (a + d))
    return a + d + j0, a + d + j1, j0, j1


@functools.lru_cache(maxsize=1)
def build_nc():
    _patch_drain_split()
    nc = bass.Bass()

    g = nc.dram_tensor("g", [IMGS_PER_CORE, H, W], F32, kind="ExternalInput")
    p = nc.dram_tensor("p", [IMGS_PER_CORE, H, W], F32, kind="ExternalInput")
    m = nc.dram_tensor("mk", [IMGS_PER_CORE, H, W], F32, kind="ExternalInput")
    out = nc.dram_tensor("acc_out", [128, N_COLS], F32, kind="ExternalOutput")

    base_mats = _make_matrices()

    with TileContext(nc) as tc:
        with (
            tc.tile_pool(name="const", bufs=1) as constp,
            tc.tile_pool(name="accp", bufs=1) as accp,
            tc.tile_pool(name="inp", bufs=3) as inp,
            tc.tile_pool(name="psum", bufs=2, space="PSUM") as psp,
            tc.tile_pool(name="work", bufs=2) as wk,
        ):
            # constants: g-side weights carry the 1.5 factor (amg = 1.5*gm),
            # p-side unscaled; packed into one tensor -> single DMA
            packed = []
            keys = []
            for kind, (sm, df) in base_mats.items():
                for side, scl in (("g", ALPHA), ("p", 1.0)):
                    for nm, arr in (
                        ("smP", scl * sm), ("smN", -scl * sm),
                        ("df", scl * df), ("df2", 2.0 * scl * df),
                    ):
                        keys.append(f"{kind}_{side}_{nm}")
                        a = np.zeros((128, 128), np.float32)
                        a[:arr.shape[0], :arr.shape[1]] = arr
                        packed.append(a)
            packed_np = np.ascontiguousarray(
                np.concatenate(packed, axis=1).astype(ml_dtypes.bfloat16))
            # tail matrices in their own tiny DMA so the (first-scheduled)
            # tail matmuls are not gated on the full constant load
            tail_idx = [i for i, kk in enumerate(keys) if kk.startswith("tail_")]
            main_idx = [i for i in range(len(keys)) if i not in tail_idx]
            tail_np = np.ascontiguousarray(
                np.concatenate([packed[i] for i in tail_idx], axis=1).astype(ml_dtypes.bfloat16))
            main_np = np.ascontiguousarray(
                np.concatenate([packed[i] for i in main_idx], axis=1).astype(ml_dtypes.bfloat16))
            consts = {}
            with tc.high_priority():
                tdram = nc.inline_tensor(tail_np, name="c_tail")
                ctail = constp.tile(list(tail_np.shape), BF16, name="ct_tail", tag="ct_tail")
                nc.sync.dma_start(out=ctail[:], in_=tdram[:])
            mdram = nc.inline_tensor(main_np, name="c_main")
            cmain = constp.tile(list(main_np.shape), BF16, name="ct_main", tag="ct_main")
            nc.sync.dma_start(out=cmain[:], in_=mdram[:])
            for j, i in enumerate(tail_idx):
                consts[keys[i]] = ctail[:, 128 * j:128 * j + 128]
            for j, i in enumerate(main_idx):
                consts[keys[i]] = cmain[:, 128 * j:128 * j + 128]

            acc = accp.tile([128, N_COLS], F32, name="acc", tag="acc")
            epsbg = accp.tile([128, 1], F32, name="epsbg", tag="epsbg")
            nc.vector.memset(epsbg[:], ALPHA * ALPHA * EPS)
            epsbp = accp.tile([128, 1], F32, name="epsbp", tag="epsbp")
            nc.vector.memset(epsbp[:], EPS)
            absmask = accp.tile([128, 1024], BF16, name="absmask", tag="absmask")
            nc.vector.memset(absmask[:].bitcast(mybir.dt.uint16), 0x7FFF)

            def do_strip(kind, K, gt, pt, mt, col):
                sqg = wk.tile([K, 2048], BF16, name="sqg", tag="sqg", bufs=3)
                bpxy = wk.tile([K, 2048], BF16, name="bpxy", tag="bpxy", bufs=3)
                prods = wk.tile([K, 2048], BF16, name="prods", tag="prods", bufs=3)
                r2 = lambda ap: ap.rearrange("p (two n) -> p two n", two=2)
                for c in (0, 1):
                    a = 512 * c
                    GXYc = psp.tile([K, 1024], F32, name="GXYc", tag="psG")
                    PXYc = psp.tile([K, 1024], F32, name="PXYc", tag="psP")
                    for side, src, ps in (("g", gt, GXYc), ("p", pt, PXYc)):
                        # x-grad -> ps[:, 0:512]; shifts L(+w), R(-w)
                        shifts_x = [(+1, f"{kind}_{side}_smN"), (-1, f"{kind}_{side}_smP")]
                        shifts_x.sort(
                            key=lambda t_: _shift_cols(a, t_[0])[3] - _shift_cols(a, t_[0])[2],
                            reverse=True)
                        for i, (d, mk_) in enumerate(shifts_x):
                            s0, s1, j0, j1 = _shift_cols(a, d)
                            nc.tensor.matmul(
                                ps[:, j0:j1],
                                lhsT=consts[mk_][0:K, 0:K],
                                rhs=src[:, s0:s1],
                                start=(i == 0), stop=(i == len(shifts_x) - 1))
                        # y-grad -> ps[:, 512:1024]
                        shifts_y = [(0, f"{kind}_{side}_df2"),
                                    (-1, f"{kind}_{side}_df"), (+1, f"{kind}_{side}_df")]
                        for i, (d, mk_) in enumerate(shifts_y):
                            s0, s1, j0, j1 = _shift_cols(a, d)
                            nc.tensor.matmul(
                                ps[:, 512 + j0:512 + j1],
                                lhsT=consts[mk_][0:K, 0:K],
                                rhs=src[:, s0:s1],
                                start=(i == 0), stop=(i == len(shifts_y) - 1))

                    # extraction into wide halves-layout tiles via 3D APs:
                    # g extracted raw (for products) + squared; p squared
                    # straight from PSUM (raw p never needs to reach SBUF)
                    nc.scalar.activation(r2(sqg)[:, :, a:a + 512], r2(GXYc), Act.Square)
                    nc.scalar.copy(r2(bpxy)[:, :, a:a + 512], r2(PXYc))
                    nc.vector.tensor_tensor(
                        r2(prods)[:, :, a:a + 512], r2(GXYc),
                        r2(bpxy)[:, :, a:a + 512], Op.mult)

                # ---- wide SBUF elementwise ----
                sqp = wk.tile([K, 2048], BF16, name="sqp", tag="sqp")
                nc.gpsimd.tensor_tensor(sqp[:, 0:1024], bpxy[:, 0:1024], bpxy[:, 0:1024], Op.mult)
                nc.vector.tensor_tensor(sqp[:, 1024:2048], bpxy[:, 1024:2048], bpxy[:, 1024:2048], Op.mult)

                e2 = wk.tile([K, 2048], BF16, name="e2", tag="e2")
                nc.vector.tensor_tensor(
                    e2[:, 0:1024], sqg[:, 0:1024], sqg[:, 1024:2048], Op.add)
                nc.vector.tensor_tensor(
                    e2[:, 1024:2048], sqp[:, 0:1024], sqp[:, 1024:2048], Op.add)

                lgp = wk.tile([K, 2048], F32, name="lgp", tag="lgp")
                nc.scalar.activation(lgp[:, 0:1024], e2[:, 0:1024], Act.Ln, bias=epsbg[0:K, :])
                nc.scalar.activation(lgp[:, 1024:2048], e2[:, 1024:2048], Act.Ln, bias=epsbp[0:K, :])
                mlg = wk.tile([K, 1024], F32, name="mlg", tag="mlg", bufs=3)
                nc.vector.tensor_tensor(mlg[:], lgp[:, 0:1024], lgp[:, 1024:2048], Op.max)
                mx = wk.tile([K, 1024], BF16, name="mx", tag="mx", bufs=3)
                nc.scalar.activation(mx[:], mlg[:], Act.Exp, scale=0.5)
                rg = wk.tile([K, 1024], BF16, name="rg", tag="rg", bufs=3)
                nc.scalar.activation(rg[:], lgp[:, 0:1024], Act.Exp, scale=-0.5)

                dotv = wk.tile([K, 1024], BF16, name="dotv", tag="dotv", bufs=3)
                nc.vector.tensor_tensor(dotv[:], prods[:, 0:1024], prods[:, 1024:2048], Op.add)
                adot = wk.tile([K, 1024], BF16, name="adot", tag="adot", bufs=3)
                nc.vector.tensor_tensor(
                    adot[:].bitcast(mybir.dt.uint16), dotv[:].bitcast(mybir.dt.uint16),
                    absmask[0:K, :].bitcast(mybir.dt.uint16), Op.bitwise_and)
                z0 = wk.tile([K, 1024], BF16, name="z0", tag="z0", bufs=3)
                nc.gpsimd.tensor_tensor(z0[:], adot[:], rg[:], Op.mult)
                tot = wk.tile([K, 1024], BF16, name="tot", tag="tot", bufs=3)
                nc.vector.tensor_tensor(tot[:], mx[:], z0[:], Op.subtract)

                dump = wk.tile([K, 1024], BF16, name="dump", tag="dump", bufs=3)
                nc.vector.scalar_tensor_tensor(
                    dump[:], tot[:], 1.0, mt[:], Op.mult, Op.mult,
                    accum_out=acc[0:K, col:col + 1])

            gt = inp.tile([32, W], BF16, name="gtt", tag="gtt")
            pt = inp.tile([32, W], BF16, name="ptt", tag="ptt")
            mt = inp.tile([32, W], F32, name="mtt", tag="mtt")
            with tc.high_priority():
                nc.gpsimd.memset(mt[:], 0)
                for img in range(IMGS_PER_CORE):
                    o = 16 * img
                    nc.gpsimd.dma_start(out=gt[o:o + 16, :], in_=g[img, 1008:1024, :])
                    nc.gpsimd.dma_start(out=pt[o:o + 16, :], in_=p[img, 1008:1024, :])
                    nc.sync.dma_start(out=mt[o:o + 15, :], in_=m[img, 1009:1024, :])
            do_strip("tail", 32, gt, pt, mt, 16)

            def mk_r0(s):
                return 0 if s == 0 else 127 + 126 * (s - 1)

            for img in range(IMGS_PER_CORE):
                for s0 in (0, 2, 4, 6):
                    # two strips per ~1MB DMA: partition p <- row 126*s + p
                    gt2 = inp.tile([128, 2 * W], BF16, name="gt2", tag="gt2")
                    pt2 = inp.tile([128, 2 * W], BF16, name="pt2", tag="pt2")
                    mt2 = inp.tile([128, 2 * W], F32, name="mt2", tag="mt2")
                    with tc.high_priority():
                        if img == 0 and s0 in (0, 2):
                            for ds in (0, 1):
                                for src_d, dst in ((g, gt2), (p, pt2)):
                                    ap = bass.AP(src_d, 126 * (s0 + ds) * W,
                                                 [[W, 128], [1, W]])
                                    nc.gpsimd.dma_start(
                                        out=dst[:, ds * W:(ds + 1) * W], in_=ap)
                        else:
                            for src_d, dst in ((g, gt2), (p, pt2)):
                                ap = bass.AP(src_d, img * H * W + 126 * s0 * W,
                                             [[W, 128], [126 * W, 2], [1, W]])
                                nc.gpsimd.dma_start(out=dst[:], in_=ap)
                    r0, r1 = mk_r0(s0), mk_r0(s0 + 1)
                    map_ = bass.AP(m, img * H * W + r0 * W,
                                   [[W, 128], [(r1 - r0) * W, 2], [1, W]])
                    nc.sync.dma_start(out=mt2[:], in_=map_)
                    for ds in (0, 1):
                        s = s0 + ds
                        kind = "top" if s == 0 else "mid"
                        do_strip(kind, 128,
                                 gt2[:, ds * W:(ds + 1) * W],
                                 pt2[:, ds * W:(ds + 1) * W],
                                 mt2[:, ds * W:(ds + 1) * W],
                                 img * 8 + s)


            nc.sync.dma_start(out=out[:], in_=acc[:])

    nc.finalize()
    return nc


def _valid_mask():
    v = np.zeros((128, N_COLS), dtype=bool)
    for img in range(IMGS_PER_CORE):
        base = img * 8
        v[0:127, base + 0] = True
        for s in range(1, 8):
            v[0:126, base + s] = True
    v[0:15, 16] = True
    v[16:31, 16] = True
    return v


def kernel(grayimg, pred, mask):
    g = np.ascontiguousarray(np.asarray(grayimg, dtype=np.float32).reshape(N_IMGS, H, W))
    p = np.ascontiguousarray(np.asarray(pred, dtype=np.float32).reshape(N_IMGS, H, W))
    mk = np.ascontiguousarray(np.asarray(mask, dtype=np.float32).reshape(N_IMGS, H, W))

    nc = build_nc()
    in_maps = []
    for c in range(N_CORES):
        sl = slice(c * IMGS_PER_CORE, (c + 1) * IMGS_PER_CORE)
        in_maps.append({"g": g[sl], "p": p[sl], "mk": mk[sl]})

    res = run_bass_kernel_spmd(nc, in_maps, core_ids=list(range(N_CORES)))

    vm = _valid_mask()
    total = 0.0
    for r in res.results:
        a = r["acc_out"].astype(np.float64)
        total += a[vm].sum()
    val = 0.5 * total / (N_IMGS * H * W)
    return np.float32(val)


# revision 45
# speedup vs baseline: 1.0674x; 1.0277x over previous
"""Trainium2 Bass kernel for nn_RefineLoss (Sobel-gradient refine loss).

Math: with gm=sqrt(|grad g|^2+eps), pm=sqrt(|grad p|^2+eps), dot=grad_g.grad_p:
  Lrefine = 0.5*(Lcos + Lmag)*mask,  Lcos = pm - |dot|/gm,
  Lmag = relu(1.5*gm - pm)  =>  Lcos + Lmag = max(1.5*gm, pm) - |dot|/gm.
Output = mean over the full tensor.

Mapping (per core: 2 images):
  - 3x3 separable Sobel convs run fully on TensorE: vertical filter as banded
    lhsT matmuls, horizontal taps via column-shifted rhs APs accumulated in
    PSUM. The g-side weights carry the 1.5 factor so 1.5*gm comes for free.
  - 128-row strips with 2-row overlap (stride 126); final 15 rows of both
    images packed into one 32-partition strip.
  - gx|gy (and px|py) land stacked in one [K,2048] PSUM tile so each
    elementwise pass covers both with one wide instruction.
  - sqrt/reciprocal via exp/ln on ScalarE (one table set, high precision):
    gm' = exp(0.5*ln(g2e)), 1/gm = exp(-0.5*ln(g2e)).
  - Per-strip masked partial sums accumulate per-partition into a [128,17]
    f32 tile (scalar_tensor_tensor accum); host does validity masking and the
    final float64 reduction.
"""

import functools

import numpy as np
import ml_dtypes

import concourse.bass as bass
import concourse.mybir as mybir
from concourse.tile import TileContext
from concourse.vector_clock import ScopedClock
from concourse.bass_utils import run_bass_kernel_spmd
from concourse.mybir import AluOpType as Op, ActivationFunctionType as Act

F32 = mybir.dt.float32
F32R = mybir.dt.float32r
BF16 = mybir.dt.bfloat16

H = W = 1024
N_IMGS = 16
N_CORES = 8
IMGS_PER_CORE = 2
EPS = 1e-6
ALPHA = 1.5
LN15 = float(np.log(1.5))

N_COLS = 17  # 2 imgs * 8 strips + 1 packed tail


def _patch_drain_split():
    """walrus in this container accepts only ONE sem-wait per instruction.
    Split every multi-wait instruction into single-wait NoOps emitted just
    before it on the same engine."""
    if getattr(TileContext, "_drain_split_patched", False):
        return

    def _patched(self, tick_clock, wait_clock):
        drain_inst = self.nc.sync.drain()
        wait_clock.add_sem_waits(
            drain_inst.ins, ScopedClock({None: tick_clock.global_clock})
        )
        si = drain_inst.ins.sync_info
        waits = list(si.on_wait or [])
        if len(waits) > 1:
            si.on_wait = waits[:1]
            for w in waits[1:]:
                nop = self.nc.sync.nop()
                nsi = nop.ins.sync_info
                if nsi is None:
                    nop.ins.sync_info = mybir.SyncInfo(on_wait=[w], on_update=[])
                else:
                    nsi.on_wait = [w]
        self.nc.all_engine_barrier()
        assert self.sems is not None
        popped = self.nc._tile_sem_poison_stack.pop()
        assert popped is self._sem_poison
        self.nc.clear_and_free_semaphores(list(self.sems.allocated().values()))
        self.nc.all_engine_barrier()

    TileContext._drain_and_barrier = _patched

    _orig_lower = TileContext._lower_ordered_insts

    def _lower_split(self, ordered):
        for bbname, insts in ordered.items():
            new = []
            for inst in insts:
                si = getattr(inst, "sync_info", None)
                waits = list(si.on_wait) if (si is not None and si.on_wait) else []
                if len(waits) > 1:
                    for i, w in enumerate(waits[:-1]):
                        new.append(mybir.InstNoOp(
                            name=f"{inst.name}_sw{i}",
                            sync_info=mybir.SyncInfo(on_wait=[w], on_update=[]),
                            bass_nofuse=True,
                            engine=inst.engine,
                        ))
                    si.on_wait = waits[-1:]
                new.append(inst)
            insts[:] = new
        return _orig_lower(self, ordered)

    TileContext._lower_ordered_insts = _lower_split
    TileContext._drain_split_patched = True


def _banded(K, M, entries):
    a = np.zeros((K, M), dtype=np.float32)
    for m in range(M):
        for off, wgt in entries:
            k = m + off
            if 0 <= k < K:
                a[k, m] = wgt
    return a


def _make_matrices():
    """Vertical-filter lhsT matrices (bf16), per strip kind.

    top   (s=0): partition k = img row k;      out m = row m,    valid m 0..126
    mid (s>=1): partition k = img row r0-1+k;  out m = row r0+m, valid m 0..125
    tail: two [16,16] diag blocks; partition kb = row 1008+kb, out mb = row
          1009+mb, valid mb 0..14.
    Returns {kind: {"sm": smooth, "df": diff}} as float32 [K, M].
    """
    mats = {}
    sm = _banded(128, 128, [(-1, 1.0), (0, 2.0), (1, 1.0)])
    df = _banded(128, 128, [(-1, 1.0), (1, -1.0)])
    sm[:, 127] = 0.0
    df[:, 127] = 0.0
    mats["top"] = (sm, df)
    sm = _banded(128, 128, [(0, 1.0), (1, 2.0), (2, 1.0)])
    df = _banded(128, 128, [(0, 1.0), (2, -1.0)])
    sm[:, 126:] = 0.0
    df[:, 126:] = 0.0
    mats["mid"] = (sm, df)
    smb = _banded(16, 16, [(0, 1.0), (1, 2.0), (2, 1.0)])
    dfb = _banded(16, 16, [(0, 1.0), (2, -1.0)])
    smb[:, 15] = 0.0
    dfb[:, 15] = 0.0
    z = np.zeros((16, 16), np.float32)
    mats["tail"] = (np.block([[smb, z], [z, smb]]), np.block([[dfb, z], [z, dfb]]))
    return mats


def _shift_cols(a, d, n=512, width=W):
    """rhs col window [a+d, a+d+n) clipped to [0,width) -> (s0, s1, j0, j1)."""
    j0 = max(0, -(a + d))
    j1 = min(n, width - (a + d))
    return a + d + j0, a + d + j1, j0, j1


@functools.lru_cache(maxsize=1)
def build_nc():
    _patch_drain_split()
    nc = bass.Bass()

    g = nc.dram_tensor("g", [IMGS_PER_CORE, H, W], F32, kind="ExternalInput")
    p = nc.dram_tensor("p", [IMGS_PER_CORE, H, W], F32, kind="ExternalInput")
    m = nc.dram_tensor("mk", [IMGS_PER_CORE, H, W], F32, kind="ExternalInput")
    out = nc.dram_tensor("acc_out", [128, N_COLS], F32, kind="ExternalOutput")

    base_mats = _make_matrices()

    with TileContext(nc) as tc:
        with (
            tc.tile_pool(name="const", bufs=1) as constp,
            tc.tile_pool(name="accp", bufs=1) as accp,
            tc.tile_pool(name="inp", bufs=3) as inp,
            tc.tile_pool(name="psum", bufs=2, space="PSUM") as psp,
            tc.tile_pool(name="work", bufs=2) as wk,
        ):
            # constants: g-side weights carry the 1.5 factor (amg = 1.5*gm),
            # p-side unscaled; packed into one tensor -> single DMA
            packed = []
            keys = []
            for kind, (sm, df) in base_mats.items():
                for side, scl in (("g", ALPHA), ("p", 1.0)):
                    for nm, arr in (
                        ("smP", scl * sm), ("smN", -scl * sm),
                        ("df", scl * df), ("df2", 2.0 * scl * df),
                    ):
                        keys.append(f"{kind}_{side}_{nm}")
                        a = np.zeros((128, 128), np.float32)
                        a[:arr.shape[0], :arr.shape[1]] = arr
                        packed.append(a)
            packed_np = np.ascontiguousarray(
                np.concatenate(packed, axis=1).astype(ml_dtypes.bfloat16))
            # tail matrices in their own tiny DMA so the (first-scheduled)
            # tail matmuls are not gated on the full constant load
            tail_idx = [i for i, kk in enumerate(keys) if kk.startswith("tail_")]
            main_idx = [i for i in range(len(keys)) if i not in tail_idx]
            tail_np = np.ascontiguousarray(
                np.concatenate([packed[i] for i in tail_idx], axis=1).astype(ml_dtypes.bfloat16))
            main_np = np.ascontiguousarray(
                np.concatenate([packed[i] for i in main_idx], axis=1).astype(ml_dtypes.bfloat16))
            consts = {}
            with tc.high_priority():
                tdram = nc.inline_tensor(tail_np, name="c_tail")
                ctail = constp.tile(list(tail_np.shape), BF16, name="ct_tail", tag="ct_tail")
                nc.sync.dma_start(out=ctail[:], in_=tdram[:])
            mdram = nc.inline_tensor(main_np, name="c_main")
            cmain = constp.tile(list(main_np.shape), BF16, name="ct_main", tag="ct_main")
            nc.sync.dma_start(out=cmain[:], in_=mdram[:])
            for j, i in enumerate(tail_idx):
                consts[keys[i]] = ctail[:, 128 * j:128 * j + 128]
            for j, i in enumerate(main_idx):
                consts[keys[i]] = cmain[:, 128 * j:128 * j + 128]

            acc = accp.tile([128, N_COLS], F32, name="acc", tag="acc")
            epsbg = accp.tile([128, 1], F32, name="epsbg", tag="epsbg")
            nc.vector.memset(epsbg[:], ALPHA * ALPHA * EPS)
            epsbp = accp.tile([128, 1], F32, name="epsbp", tag="epsbp")
            nc.vector.memset(epsbp[:], EPS)
            absmask = accp.tile([128, 1024], BF16, name="absmask", tag="absmask")
            nc.vector.memset(absmask[:].bitcast(mybir.dt.uint16), 0x7FFF)

            def do_strip(kind, K, gt, pt, mt, col):
                sqg = wk.tile([K, 2048], BF16, name="sqg", tag="sqg", bufs=3)
                bpxy = wk.tile([K, 2048], BF16, name="bpxy", tag="bpxy", bufs=3)
                prods = wk.tile([K, 2048], BF16, name="prods", tag="prods", bufs=3)
                r2 = lambda ap: ap.rearrange("p (two n) -> p two n", two=2)
                for c in (0, 1):
                    a = 512 * c
                    GXYc = psp.tile([K, 1024], F32, name="GXYc", tag="psG")
                    PXYc = psp.tile([K, 1024], F32, name="PXYc", tag="psP")
                    for side, src, ps in (("g", gt, GXYc), ("p", pt, PXYc)):
                        # x-grad -> ps[:, 0:512]; shifts L(+w), R(-w)
                        shifts_x = [(+1, f"{kind}_{side}_smN"), (-1, f"{kind}_{side}_smP")]
                        shifts_x.sort(
                            key=lambda t_: _shift_cols(a, t_[0])[3] - _shift_cols(a, t_[0])[2],
                            reverse=True)
                        for i, (d, mk_) in enumerate(shifts_x):
                            s0, s1, j0, j1 = _shift_cols(a, d)
                            nc.tensor.matmul(
                                ps[:, j0:j1],
                                lhsT=consts[mk_][0:K, 0:K],
                                rhs=src[:, s0:s1],
                                start=(i == 0), stop=(i == len(shifts_x) - 1))
                        # y-grad -> ps[:, 512:1024]
                        shifts_y = [(0, f"{kind}_{side}_df2"),
                                    (-1, f"{kind}_{side}_df"), (+1, f"{kind}_{side}_df")]
                        for i, (d, mk_) in enumerate(shifts_y):
                            s0, s1, j0, j1 = _shift_cols(a, d)
                            nc.tensor.matmul(
                                ps[:, 512 + j0:512 + j1],
                                lhsT=consts[mk_][0:K, 0:K],
                                rhs=src[:, s0:s1],
                                start=(i == 0), stop=(i == len(shifts_y) - 1))

                    # extraction into wide halves-layout tiles via 3D APs:
                    # g extracted raw (for products) + squared; p squared
                    # straight from PSUM (raw p never needs to reach SBUF)
                    nc.scalar.activation(r2(sqg)[:, :, a:a + 512], r2(GXYc), Act.Square)
                    nc.scalar.copy(r2(bpxy)[:, :, a:a + 512], r2(PXYc))
                    nc.vector.tensor_tensor(
                        r2(prods)[:, :, a:a + 512], r2(GXYc),
                        r2(bpxy)[:, :, a:a + 512], Op.mult)

                # ---- wide SBUF elementwise ----
                sqp = wk.tile([K, 2048], BF16, name="sqp", tag="sqp")
                nc.gpsimd.tensor_tensor(sqp[:, 0:1024], bpxy[:, 0:1024], bpxy[:, 0:1024], Op.mult)
                nc.vector.tensor_tensor(sqp[:, 1024:2048], bpxy[:, 1024:2048], bpxy[:, 1024:2048], Op.mult)

                e2 = wk.tile([K, 2048], BF16, name="e2", tag="e2")
                nc.vector.tensor_tensor(
                    e2[:, 0:1024], sqg[:, 0:1024], sqg[:, 1024:2048], Op.add)
                nc.vector.tensor_tensor(
                    e2[:, 1024:2048], sqp[:, 0:1024], sqp[:, 1024:2048], Op.add)

                lgp = wk.tile([K, 2048], F32, name="lgp", tag="lgp")
                nc.scalar.activation(lgp[:, 0:1024], e2[:, 0:1024], Act.Ln, bias=epsbg[0:K, :])
                nc.scalar.activation(lgp[:, 1024:2048], e2[:, 1024:2048], Act.Ln, bias=epsbp[0:K, :])
                mlg = wk.tile([K, 1024], F32, name="mlg", tag="mlg", bufs=3)
                nc.vector.tensor_tensor(mlg[:], lgp[:, 0:1024], lgp[:, 1024:2048], Op.max)
                mx = wk.tile([K, 1024], BF16, name="mx", tag="mx", bufs=3)
                nc.scalar.activation(mx[:], mlg[:], Act.Exp, scale=0.5)
                rg = wk.tile([K, 1024], BF16, name="rg", tag="rg", bufs=3)
                nc.scalar.activation(rg[:], lgp[:, 0:1024], Act.Exp, scale=-0.5)

                dotv = wk.tile([K, 1024], BF16, name="dotv", tag="dotv", bufs=3)
                nc.vector.tensor_tensor(dotv[:], prods[:, 0:1024], prods[:, 1024:2048], Op.add)
                adot = wk.tile([K, 1024], BF16, name="adot", tag="adot", bufs=3)
                nc.vector.tensor_tensor(
                    adot[:].bitcast(mybir.dt.uint16), dotv[:].bitcast(mybir.dt.uint16),
                    absmask[0:K, :].bitcast(mybir.dt.uint16), Op.bitwise_and)
                z0 = wk.tile([K, 1024], BF16, name="z0", tag="z0", bufs=3)
                nc.gpsimd.tensor_tensor(z0[:], adot[:], rg[:], Op.mult)
                tot = wk.tile([K, 1024], BF16, name="tot", tag="tot", bufs=3)
                nc.vector.tensor_tensor(tot[:], mx[:], z0[:], Op.subtract)

                dump = wk.tile([K, 1024], BF16, name="dump", tag="dump", bufs=3)
                nc.vector.scalar_tensor_tensor(
                    dump[:], tot[:], 1.0, mt[:], Op.mult, Op.mult,
                    accum_out=acc[0:K, col:col + 1])

            gt = inp.tile([32, W], BF16, name="gtt", tag="gtt")
            pt = inp.tile([32, W], BF16, name="ptt", tag="ptt")
            mt = inp.tile([32, W], BF16, name="mtt", tag="mtt")
            with tc.high_priority():
                nc.gpsimd.memset(mt[:], 0)
                for img in range(IMGS_PER_CORE):
                    o = 16 * img
                    nc.gpsimd.dma_start(out=gt[o:o + 16, :], in_=g[img, 1008:1024, :])
                    nc.gpsimd.dma_start(out=pt[o:o + 16, :], in_=p[img, 1008:1024, :])
                    nc.gpsimd.dma_start(out=mt[o:o + 15, :], in_=m[img, 1009:1024, :])
            do_strip("tail", 32, gt, pt, mt, 16)

            def mk_r0(s):
                return 0 if s == 0 else 127 + 126 * (s - 1)

            for img in range(IMGS_PER_CORE):
                for s0 in (0, 2, 4, 6):
                    # two strips per ~1MB DMA: partition p <- row 126*s + p
                    gt2 = inp.tile([128, 2 * W], BF16, name="gt2", tag="gt2")
                    pt2 = inp.tile([128, 2 * W], BF16, name="pt2", tag="pt2")
                    mt2 = inp.tile([128, 2 * W], BF16, name="mt2", tag="mt2")
                    with tc.high_priority():
                        if img == 0 and s0 in (0, 2):
                            for ds in (0, 1):
                                for src_d, dst in ((g, gt2), (p, pt2)):
                                    ap = bass.AP(src_d, 126 * (s0 + ds) * W,
                                                 [[W, 128], [1, W]])
                                    nc.gpsimd.dma_start(
                                        out=dst[:, ds * W:(ds + 1) * W], in_=ap)
                        else:
                            for src_d, dst in ((g, gt2), (p, pt2)):
                                ap = bass.AP(src_d, img * H * W + 126 * s0 * W,
                                             [[W, 128], [126 * W, 2], [1, W]])
                                nc.gpsimd.dma_start(out=dst[:], in_=ap)
                    r0, r1 = mk_r0(s0), mk_r0(s0 + 1)
                    map_ = bass.AP(m, img * H * W + r0 * W,
                                   [[W, 128], [(r1 - r0) * W, 2], [1, W]])
                    nc.gpsimd.dma_start(out=mt2[:], in_=map_)
                    for ds in (0, 1):
                        s = s0 + ds
                        kind = "top" if s == 0 else "mid"
                        do_strip(kind, 128,
                                 gt2[:, ds * W:(ds + 1) * W],
                                 pt2[:, ds * W:(ds + 1) * W],
                                 mt2[:, ds * W:(ds + 1) * W],
                                 img * 8 + s)


            nc.sync.dma_start(out=out[:], in_=acc[:])

    nc.finalize()
    return nc


def _valid_mask():
    v = np.zeros((128, N_COLS), dtype=bool)
    for img in range(IMGS_PER_CORE):
        base = img * 8
        v[0:127, base + 0] = True
        for s in range(1, 8):
            v[0:126, base + s] = True
    v[0:15, 16] = True
    v[16:31, 16] = True
    return v


def kernel(grayimg, pred, mask):
    g = np.ascontiguousarray(np.asarray(grayimg, dtype=np.float32).reshape(N_IMGS, H, W))
    p = np.ascontiguousarray(np.asarray(pred, dtype=np.float32).reshape(N_IMGS, H, W))
    mk = np.ascontiguousarray(np.asarray(mask, dtype=np.float32).reshape(N_IMGS, H, W))

    nc = build_nc()
    in_maps = []
    for c in range(N_CORES):
        sl = slice(c * IMGS_PER_CORE, (c + 1) * IMGS_PER_CORE)
        in_maps.append({"g": g[sl], "p": p[sl], "mk": mk[sl]})

    res = run_bass_kernel_spmd(nc, in_maps, core_ids=list(range(N_CORES)))

    vm = _valid_mask()
    total = 0.0
    for r in res.results:
        a = r["acc_out"].astype(np.float64)
        total += a[vm].sum()
    val = 0.5 * total / (N_IMGS * H * W)
    return np.float32(val)


# revision 49
# speedup vs baseline: 1.1555x; 1.0825x over previous
"""Trainium2 Bass kernel for nn_RefineLoss (Sobel-gradient refine loss).

Math: with gm=sqrt(|grad g|^2+eps), pm=sqrt(|grad p|^2+eps), dot=grad_g.grad_p:
  Lrefine = 0.5*(Lcos + Lmag)*mask,  Lcos = pm - |dot|/gm,
  Lmag = relu(1.5*gm - pm)  =>  Lcos + Lmag = max(1.5*gm, pm) - |dot|/gm.
Output = mean over the full tensor.

Mapping (per core: 2 images):
  - 3x3 separable Sobel convs run fully on TensorE: vertical filter as banded
    lhsT matmuls, horizontal taps via column-shifted rhs APs accumulated in
    PSUM. The g-side weights carry the 1.5 factor so 1.5*gm comes for free.
  - 128-row strips with 2-row overlap (stride 126); final 15 rows of both
    images packed into one 32-partition strip.
  - gx|gy (and px|py) land stacked in one [K,2048] PSUM tile so each
    elementwise pass covers both with one wide instruction.
  - sqrt/reciprocal via exp/ln on ScalarE (one table set, high precision):
    gm' = exp(0.5*ln(g2e)), 1/gm = exp(-0.5*ln(g2e)).
  - Per-strip masked partial sums accumulate per-partition into a [128,17]
    f32 tile (scalar_tensor_tensor accum); host does validity masking and the
    final float64 reduction.
"""

import functools

import numpy as np
import ml_dtypes

import concourse.bass as bass
import concourse.mybir as mybir
from concourse.tile import TileContext
from concourse.vector_clock import ScopedClock
from concourse.bass_utils import run_bass_kernel_spmd
from concourse.mybir import AluOpType as Op, ActivationFunctionType as Act

F32 = mybir.dt.float32
F32R = mybir.dt.float32r
BF16 = mybir.dt.bfloat16

H = W = 1024
N_IMGS = 16
N_CORES = 8
IMGS_PER_CORE = 2
EPS = 1e-6
ALPHA = 1.5
LN15 = float(np.log(1.5))

N_COLS = 17  # 2 imgs * 8 strips + 1 packed tail


def _patch_drain_split():
    """walrus in this container accepts only ONE sem-wait per instruction.
    Split every multi-wait instruction into single-wait NoOps emitted just
    before it on the same engine."""
    if getattr(TileContext, "_drain_split_patched", False):
        return

    def _patched(self, tick_clock, wait_clock):
        drain_inst = self.nc.sync.drain()
        wait_clock.add_sem_waits(
            drain_inst.ins, ScopedClock({None: tick_clock.global_clock})
        )
        si = drain_inst.ins.sync_info
        waits = list(si.on_wait or [])
        if len(waits) > 1:
            si.on_wait = waits[:1]
            for w in waits[1:]:
                nop = self.nc.sync.nop()
                nsi = nop.ins.sync_info
                if nsi is None:
                    nop.ins.sync_info = mybir.SyncInfo(on_wait=[w], on_update=[])
                else:
                    nsi.on_wait = [w]
        self.nc.all_engine_barrier()
        assert self.sems is not None
        popped = self.nc._tile_sem_poison_stack.pop()
        assert popped is self._sem_poison
        self.nc.clear_and_free_semaphores(list(self.sems.allocated().values()))
        self.nc.all_engine_barrier()

    TileContext._drain_and_barrier = _patched

    _orig_lower = TileContext._lower_ordered_insts

    def _lower_split(self, ordered):
        for bbname, insts in ordered.items():
            new = []
            for inst in insts:
                si = getattr(inst, "sync_info", None)
                waits = list(si.on_wait) if (si is not None and si.on_wait) else []
                if len(waits) > 1:
                    for i, w in enumerate(waits[:-1]):
                        new.append(mybir.InstNoOp(
                            name=f"{inst.name}_sw{i}",
                            sync_info=mybir.SyncInfo(on_wait=[w], on_update=[]),
                            bass_nofuse=True,
                            engine=inst.engine,
                        ))
                    si.on_wait = waits[-1:]
                new.append(inst)
            insts[:] = new
        return _orig_lower(self, ordered)

    TileContext._lower_ordered_insts = _lower_split
    TileContext._drain_split_patched = True


def _banded(K, M, entries):
    a = np.zeros((K, M), dtype=np.float32)
    for m in range(M):
        for off, wgt in entries:
            k = m + off
            if 0 <= k < K:
                a[k, m] = wgt
    return a


def _make_matrices():
    """Vertical-filter lhsT matrices (bf16), per strip kind.

    top   (s=0): partition k = img row k;      out m = row m,    valid m 0..126
    mid (s>=1): partition k = img row r0-1+k;  out m = row r0+m, valid m 0..125
    tail: two [16,16] diag blocks; partition kb = row 1008+kb, out mb = row
          1009+mb, valid mb 0..14.
    Returns {kind: {"sm": smooth, "df": diff}} as float32 [K, M].
    """
    mats = {}
    sm = _banded(128, 128, [(-1, 1.0), (0, 2.0), (1, 1.0)])
    df = _banded(128, 128, [(-1, 1.0), (1, -1.0)])
    sm[:, 127] = 0.0
    df[:, 127] = 0.0
    mats["top"] = (sm, df)
    sm = _banded(128, 128, [(0, 1.0), (1, 2.0), (2, 1.0)])
    df = _banded(128, 128, [(0, 1.0), (2, -1.0)])
    sm[:, 126:] = 0.0
    df[:, 126:] = 0.0
    mats["mid"] = (sm, df)
    smb = _banded(16, 16, [(0, 1.0), (1, 2.0), (2, 1.0)])
    dfb = _banded(16, 16, [(0, 1.0), (2, -1.0)])
    smb[:, 15] = 0.0
    dfb[:, 15] = 0.0
    z = np.zeros((16, 16), np.float32)
    mats["tail"] = (np.block([[smb, z], [z, smb]]), np.block([[dfb, z], [z, dfb]]))
    return mats


def _shift_cols(a, d, n=512, width=W):
    """rhs col window [a+d, a+d+n) clipped to [0,width) -> (s0, s1, j0, j1)."""
    j0 = max(0, -(a + d))
    j1 = min(n, width - (a + d))
    return a + d + j0, a + d + j1, j0, j1


@functools.lru_cache(maxsize=1)
def build_nc():
    _patch_drain_split()
    nc = bass.Bass()

    g = nc.dram_tensor("g", [IMGS_PER_CORE, H, W], F32, kind="ExternalInput")
    p = nc.dram_tensor("p", [IMGS_PER_CORE, H, W], F32, kind="ExternalInput")
    m = nc.dram_tensor("mk", [IMGS_PER_CORE, H, W], F32, kind="ExternalInput")
    out = nc.dram_tensor("acc_out", [128, N_COLS], F32, kind="ExternalOutput")

    base_mats = _make_matrices()

    with TileContext(nc) as tc:
        with (
            tc.tile_pool(name="const", bufs=1) as constp,
            tc.tile_pool(name="accp", bufs=1) as accp,
            tc.tile_pool(name="inp", bufs=3) as inp,
            tc.tile_pool(name="psum", bufs=2, space="PSUM") as psp,
            tc.tile_pool(name="work", bufs=2) as wk,
        ):
            # constants: g-side weights carry the 1.5 factor (amg = 1.5*gm),
            # p-side unscaled; packed into one tensor -> single DMA
            packed = []
            keys = []
            for kind, (sm, df) in base_mats.items():
                for side, scl in (("g", ALPHA), ("p", 1.0)):
                    for nm, arr in (
                        ("smP", scl * sm), ("smN", -scl * sm),
                        ("df", scl * df), ("df2", 2.0 * scl * df),
                    ):
                        keys.append(f"{kind}_{side}_{nm}")
                        a = np.zeros((128, 128), np.float32)
                        a[:arr.shape[0], :arr.shape[1]] = arr
                        packed.append(a)
            packed_np = np.ascontiguousarray(
                np.concatenate(packed, axis=1).astype(ml_dtypes.bfloat16))
            # tail matrices in their own tiny DMA so the (first-scheduled)
            # tail matmuls are not gated on the full constant load
            tail_idx = [i for i, kk in enumerate(keys) if kk.startswith("tail_")]
            main_idx = [i for i in range(len(keys)) if i not in tail_idx]
            tail_np = np.ascontiguousarray(
                np.concatenate([packed[i] for i in tail_idx], axis=1).astype(ml_dtypes.bfloat16))
            main_np = np.ascontiguousarray(
                np.concatenate([packed[i] for i in main_idx], axis=1).astype(ml_dtypes.bfloat16))
            consts = {}
            with tc.high_priority():
                tdram = nc.inline_tensor(tail_np, name="c_tail")
                ctail = constp.tile(list(tail_np.shape), BF16, name="ct_tail", tag="ct_tail")
                nc.sync.dma_start(out=ctail[:], in_=tdram[:])
            mdram = nc.inline_tensor(main_np, name="c_main")
            cmain = constp.tile(list(main_np.shape), BF16, name="ct_main", tag="ct_main")
            nc.sync.dma_start(out=cmain[:], in_=mdram[:])
            for j, i in enumerate(tail_idx):
                consts[keys[i]] = ctail[:, 128 * j:128 * j + 128]
            for j, i in enumerate(main_idx):
                consts[keys[i]] = cmain[:, 128 * j:128 * j + 128]

            acc = accp.tile([128, N_COLS], F32, name="acc", tag="acc")
            epsbg = accp.tile([128, 1], F32, name="epsbg", tag="epsbg")
            nc.vector.memset(epsbg[:], ALPHA * ALPHA * EPS)
            epsbp = accp.tile([128, 1], F32, name="epsbp", tag="epsbp")
            nc.vector.memset(epsbp[:], EPS)
            absmask = accp.tile([128, 1024], BF16, name="absmask", tag="absmask")
            nc.vector.memset(absmask[:].bitcast(mybir.dt.uint16), 0x7FFF)

            def do_strip(kind, K, gt, pt, mt, col):
                sqg = wk.tile([K, 2048], BF16, name="sqg", tag="sqg", bufs=3)
                bpxy = wk.tile([K, 2048], BF16, name="bpxy", tag="bpxy", bufs=3)
                prods = wk.tile([K, 2048], BF16, name="prods", tag="prods", bufs=3)
                r2 = lambda ap: ap.rearrange("p (two n) -> p two n", two=2)
                for c in (0, 1):
                    a = 512 * c
                    GXYc = psp.tile([K, 1024], F32, name="GXYc", tag="psG")
                    PXYc = psp.tile([K, 1024], F32, name="PXYc", tag="psP")
                    for side, src, ps in (("g", gt, GXYc), ("p", pt, PXYc)):
                        # x-grad -> ps[:, 0:512]; shifts L(+w), R(-w)
                        shifts_x = [(+1, f"{kind}_{side}_smN"), (-1, f"{kind}_{side}_smP")]
                        shifts_x.sort(
                            key=lambda t_: _shift_cols(a, t_[0])[3] - _shift_cols(a, t_[0])[2],
                            reverse=True)
                        for i, (d, mk_) in enumerate(shifts_x):
                            s0, s1, j0, j1 = _shift_cols(a, d)
                            nc.tensor.matmul(
                                ps[:, j0:j1],
                                lhsT=consts[mk_][0:K, 0:K],
                                rhs=src[:, s0:s1],
                                start=(i == 0), stop=(i == len(shifts_x) - 1))
                        # y-grad -> ps[:, 512:1024]
                        shifts_y = [(0, f"{kind}_{side}_df2"),
                                    (-1, f"{kind}_{side}_df"), (+1, f"{kind}_{side}_df")]
                        for i, (d, mk_) in enumerate(shifts_y):
                            s0, s1, j0, j1 = _shift_cols(a, d)
                            nc.tensor.matmul(
                                ps[:, 512 + j0:512 + j1],
                                lhsT=consts[mk_][0:K, 0:K],
                                rhs=src[:, s0:s1],
                                start=(i == 0), stop=(i == len(shifts_y) - 1))

                    # extraction into wide halves-layout tiles via 3D APs:
                    # g extracted raw (for products) + squared; p squared
                    # straight from PSUM (raw p never needs to reach SBUF)
                    nc.scalar.activation(r2(sqg)[:, :, a:a + 512], r2(GXYc), Act.Square)
                    nc.scalar.copy(r2(bpxy)[:, :, a:a + 512], r2(PXYc))
                    nc.vector.tensor_tensor(
                        r2(prods)[:, :, a:a + 512], r2(GXYc),
                        r2(bpxy)[:, :, a:a + 512], Op.mult)

                # ---- wide SBUF elementwise ----
                sqp = wk.tile([K, 2048], BF16, name="sqp", tag="sqp")
                nc.gpsimd.tensor_tensor(sqp[:, 0:1024], bpxy[:, 0:1024], bpxy[:, 0:1024], Op.mult)
                nc.vector.tensor_tensor(sqp[:, 1024:2048], bpxy[:, 1024:2048], bpxy[:, 1024:2048], Op.mult)

                e2 = wk.tile([K, 2048], BF16, name="e2", tag="e2")
                nc.vector.tensor_tensor(
                    e2[:, 0:1024], sqg[:, 0:1024], sqg[:, 1024:2048], Op.add)
                nc.vector.tensor_tensor(
                    e2[:, 1024:2048], sqp[:, 0:1024], sqp[:, 1024:2048], Op.add)

                lgp = wk.tile([K, 2048], F32, name="lgp", tag="lgp")
                nc.scalar.activation(lgp[:, 0:1024], e2[:, 0:1024], Act.Ln, bias=epsbg[0:K, :])
                nc.scalar.activation(lgp[:, 1024:2048], e2[:, 1024:2048], Act.Ln, bias=epsbp[0:K, :])
                mlg = wk.tile([K, 1024], F32, name="mlg", tag="mlg", bufs=3)
                nc.vector.tensor_tensor(mlg[:], lgp[:, 0:1024], lgp[:, 1024:2048], Op.max)
                mx = wk.tile([K, 1024], BF16, name="mx", tag="mx", bufs=3)
                nc.scalar.activation(mx[:], mlg[:], Act.Exp, scale=0.5)
                rg = wk.tile([K, 1024], BF16, name="rg", tag="rg", bufs=3)
                nc.scalar.activation(rg[:], lgp[:, 0:1024], Act.Exp, scale=-0.5)

                dotv = wk.tile([K, 1024], BF16, name="dotv", tag="dotv", bufs=3)
                nc.vector.tensor_tensor(dotv[:], prods[:, 0:1024], prods[:, 1024:2048], Op.add)
                adot = wk.tile([K, 1024], BF16, name="adot", tag="adot", bufs=3)
                nc.vector.tensor_tensor(
                    adot[:].bitcast(mybir.dt.uint16), dotv[:].bitcast(mybir.dt.uint16),
                    absmask[0:K, :].bitcast(mybir.dt.uint16), Op.bitwise_and)
                z0 = wk.tile([K, 1024], BF16, name="z0", tag="z0", bufs=3)
                nc.gpsimd.tensor_tensor(z0[:], adot[:], rg[:], Op.mult)
                tot = wk.tile([K, 1024], BF16, name="tot", tag="tot", bufs=3)
                nc.vector.tensor_tensor(tot[:], mx[:], z0[:], Op.subtract)

                dump = wk.tile([K, 1024], BF16, name="dump", tag="dump", bufs=3)
                nc.vector.scalar_tensor_tensor(
                    dump[:], tot[:], 1.0, mt[:], Op.mult, Op.mult,
                    accum_out=acc[0:K, col:col + 1])

            gt = inp.tile([32, W], BF16, name="gtt", tag="gtt")
            pt = inp.tile([32, W], BF16, name="ptt", tag="ptt")
            mt = inp.tile([32, W], BF16, name="mtt", tag="mtt")
            with tc.high_priority(offset=10**6):
                nc.vector.memset(mt[:], 0)
                for img in range(IMGS_PER_CORE):
                    o = 16 * img
                    nc.gpsimd.dma_start(out=gt[o:o + 16, :], in_=g[img, 1008:1024, :])
                    nc.gpsimd.dma_start(out=pt[o:o + 16, :], in_=p[img, 1008:1024, :])
                    nc.gpsimd.dma_start(out=mt[o:o + 15, :], in_=m[img, 1009:1024, :])
            do_strip("tail", 32, gt, pt, mt, 16)

            def mk_r0(s):
                return 0 if s == 0 else 127 + 126 * (s - 1)

            for img in range(IMGS_PER_CORE):
                for s0 in (0, 2, 4, 6):
                    # two strips per ~1MB DMA: partition p <- row 126*s + p
                    gt2 = inp.tile([128, 2 * W], BF16, name="gt2", tag="gt2")
                    pt2 = inp.tile([128, 2 * W], BF16, name="pt2", tag="pt2")
                    mt2 = inp.tile([128, 2 * W], BF16, name="mt2", tag="mt2")
                    with tc.high_priority():
                        if img == 0 and s0 in (0, 2):
                            for ds in (0, 1):
                                for src_d, dst in ((g, gt2), (p, pt2)):
                                    ap = bass.AP(src_d, 126 * (s0 + ds) * W,
                                                 [[W, 128], [1, W]])
                                    nc.gpsimd.dma_start(
                                        out=dst[:, ds * W:(ds + 1) * W], in_=ap)
                        else:
                            for src_d, dst in ((g, gt2), (p, pt2)):
                                ap = bass.AP(src_d, img * H * W + 126 * s0 * W,
                                             [[W, 128], [126 * W, 2], [1, W]])
                                nc.gpsimd.dma_start(out=dst[:], in_=ap)
                    r0, r1 = mk_r0(s0), mk_r0(s0 + 1)
                    map_ = bass.AP(m, img * H * W + r0 * W,
                                   [[W, 128], [(r1 - r0) * W, 2], [1, W]])
                    nc.gpsimd.dma_start(out=mt2[:], in_=map_)
                    for ds in (0, 1):
                        s = s0 + ds
                        kind = "top" if s == 0 else "mid"
                        do_strip(kind, 128,
                                 gt2[:, ds * W:(ds + 1) * W],
                                 pt2[:, ds * W:(ds + 1) * W],
                                 mt2[:, ds * W:(ds + 1) * W],
                                 img * 8 + s)


            nc.sync.dma_start(out=out[:], in_=acc[:])

    nc.finalize()
    return nc


def _valid_mask():
    v = np.zeros((128, N_COLS), dtype=bool)
    for img in range(IMGS_PER_CORE):
        base = img * 8
        v[0:127, base + 0] = True
        for s in range(1, 8):
            v[0:126, base + s] = True
    v[0:15, 16] = True
    v[16:31, 16] = True
    return v


def kernel(grayimg, pred, mask):
    g = np.ascontiguousarray(np.asarray(grayimg, dtype=np.float32).reshape(N_IMGS, H, W))
    p = np.ascontiguousarray(np.asarray(pred, dtype=np.float32).reshape(N_IMGS, H, W))
    mk = np.ascontiguousarray(np.asarray(mask, dtype=np.float32).reshape(N_IMGS, H, W))

    nc = build_nc()
    in_maps = []
    for c in range(N_CORES):
        sl = slice(c * IMGS_PER_CORE, (c + 1) * IMGS_PER_CORE)
        in_maps.append({"g": g[sl], "p": p[sl], "mk": mk[sl]})

    res = run_bass_kernel_spmd(nc, in_maps, core_ids=list(range(N_CORES)))

    vm = _valid_mask()
    total = 0.0
    for r in res.results:
        a = r["acc_out"].astype(np.float64)
        total += a[vm].sum()
    val = 0.5 * total / (N_IMGS * H * W)
    return np.float32(val)
